# revision 1
# baseline (speedup 1.0000x reference)
"""GCN encoder (GCNConv + PReLU) as a Bass/Tile kernel on 8 Trainium2 NeuronCores.

Math (matches PyG GCNConv with self-loops + symmetric norm, then PReLU):
    deg[i]  = in-degree of i over dst (+1 self loop)
    dinv    = 1/sqrt(deg)
    agg[d]  = sum_{e:(s->d)} dinv[s]*dinv[d] * x[s] + dinv[d]^2 * x[d]
    out     = PReLU(agg @ W.T + bias)

Distribution: dst-node sharding, core k owns nodes [k*6250, (k+1)*6250).

Per-core pipeline (dst-blocks of 128 nodes):
  - non-self edges are grouped by (dst-block, src-half) on the host and packed
    into 128-edge chunks; src rows are fetched with `dma_gather` (int16
    indices => x is split into two 25000-row halves). Gathers of GBLK
    consecutive blocks are merged per instruction and rotated over 4 SWDGE
    queues so descriptor generation and SDMA drains pipeline.
  - a per-chunk selection matrix Msel[e, d] = (d == dst_local[e]) * norm[e]
    (one fused DVE op from an iota tile) turns the scatter-add into a single
    PE matmul per chunk, accumulating A[d, c] += Msel[e, d]^T @ gx[e, c].
  - the self-loop term is added as a dense, host-prescaled tile via one
    extra identity matmul: A[d, :] += I^T @ (dinv^2 * x)[d, :].
  - A is transposed with the PE (128x128 via identity) so the weight matmul
    H[n, h] = A^T[c, n]^T @ W^T[c, h] + 1^T @ bias accumulates in PSUM.
  - PReLU = max(H, alpha*H): scalar-engine copy with scale=alpha, then a
    vector max against PSUM (exact for 0 <= alpha <= 1; general fallback
    uses relu(H)*(1-alpha) + alpha*H).

Dtype knobs (env):
  GCN_SC_DT  = f32 | f32r | bf16   scatter path (gather + Msel + edge matmul)
  GCN_FIN_DT = f32 | f32r          weight matmul path
f32r/bf16 run the PE at 1 cycle/row instead of fp32's 4 (PSUM accumulation is
fp32 in all modes); bf16 additionally halves the gather DMA traffic.
"""

import os
import numpy as np
from contextlib import ExitStack

import concourse.bass as bass
import concourse.tile as tile
from concourse import bacc, mybir, bass_utils
from concourse.masks import make_identity

# Problem shape (fixed by the harness contract).
N_NODES = 50000
N_EDGES = 400000
IN_CH = 256
HID = 512
NCORES = 8
NPC = N_NODES // NCORES  # dst nodes owned per core
P = 128

F32 = mybir.dt.float32
BF16 = mybir.dt.bfloat16
# blocks whose gathers are merged into one dma_gather pair (lo/hi)
GBLK = int(os.environ.get("GCN_GBLK", "2"))
# of every 8 Msel builds, this many go to the scalar engine (rest on vector)
MSACT = int(os.environ.get("GCN_MSACT", "0"))


def _preprocess(edge_index, n_nodes=N_NODES, ncores=NCORES):
    """Group non-self edges by (core, dst-block, src-half); pack into 128-edge
    chunks (counts maxed over cores so all cores share one program).

    Returns (klo, khi, idx16, dstl, nrm, dinv):
      klo/khi: per-block chunk counts for the lo/hi gathers (compile-time)
      idx16:   [ncores, 128, 8*tot] int16 gather indices (16-wrap, 8x tiled)
      dstl:    [ncores, 128, tot] f32 dst-local-in-block per edge slot
      nrm:     [ncores, 128, tot] f32 edge norm (0 on padded slots)
      dinv:    [n_nodes] f32 1/sqrt(deg)
    """
    dblk = P
    npc = n_nodes // ncores
    half = n_nodes // 2
    src = np.asarray(edge_index[0]).astype(np.int64).ravel()
    dst = np.asarray(edge_index[1]).astype(np.int64).ravel()
    deg = np.bincount(dst, minlength=n_nodes).astype(np.float32) + 1.0
    dinv = (1.0 / np.sqrt(deg)).astype(np.float32)
    n_all = dinv[src] * dinv[dst]

    core = dst // npc
    dloc = dst - core * npc
    blk = dloc // dblk
    bpc = (npc + dblk - 1) // dblk
    hi = (src >= half).astype(np.int64)

    key = (core * bpc + blk) * 2 + hi
    nkeys = ncores * bpc * 2
    counts = np.bincount(key, minlength=nkeys).reshape(ncores, bpc, 2)
    cmax = counts.max(axis=0)  # [bpc, 2]
    klo = [max(1, -(-int(c) // P)) if c > 0 else 0 for c in cmax[:, 0]]
    khi = [max(1, -(-int(c) // P)) if c > 0 else 0 for c in cmax[:, 1]]
    kblk = [a + b for a, b in zip(klo, khi)]
    chunk_off = np.zeros(bpc + 1, np.int64)
    chunk_off[1:] = np.cumsum(kblk)
    tot = int(chunk_off[-1])

    order = np.argsort(key, kind="stable")
    key_sorted = key[order]
    grp_start = np.zeros(nkeys + 1, np.int64)
    grp_start[1:] = np.cumsum(counts.ravel())
    rank = np.arange(len(key_sorted)) - grp_start[key_sorted]

    # chunk layout groups GBLK consecutive blocks per gather pair:
    # [lo(b0) lo(b1) .. | hi(b0) hi(b1) ..] per group, groups consecutive
    segbase = np.zeros((bpc, 2), np.int64)
    off = 0
    for g0 in range(0, bpc, GBLK):
        blocks = range(g0, min(g0 + GBLK, bpc))
        for b in blocks:
            segbase[b, 0] = off
            off += klo[b]
        for b in blocks:
            segbase[b, 1] = off
            off += khi[b]
    assert off == tot

    ob, oh, oc = blk[order], hi[order], core[order]
    base = segbase[ob, oh]
    ck = base + rank // P
    pp = rank % P

    dstl = np.zeros((ncores, P, tot), np.float32)
    nrm = np.zeros((ncores, P, tot), np.float32)
    dstl[oc, pp, ck] = (dloc[order] - ob * dblk).astype(np.float32)
    nrm[oc, pp, ck] = n_all[order]

    s16 = (src[order] - oh * half).astype(np.int16)
    col = 8 * base + (rank // 16)
    row = rank % 16
    idx16 = np.zeros((ncores, 16, 8 * tot), np.int16)
    idx16[oc, row, col] = s16
    idx16 = np.tile(idx16, (1, 8, 1))
    return klo, khi, idx16, dstl, nrm, dinv


def _build_program(
    klo,
    khi,
    alpha,
    sc_dt=F32,
    sc_mm_dt=None,
    fin_mm_dt=None,
    n_nodes=N_NODES,
    ncores=NCORES,
    in_ch=IN_CH,
    hid=HID,
):
    """Build the per-core Bass program (identical across cores).

    sc_dt: storage dtype of gather/Msel tiles (F32 or BF16).
    sc_mm_dt: dtype the scatter matmul sees (defaults to sc_dt; use
        mybir.dt.float32r with sc_dt=F32 for fast near-fp32 matmuls).
    fin_mm_dt: dtype of the weight matmul (F32 or float32r).
    """
    dblk = P
    npc = n_nodes // ncores
    half = n_nodes // 2
    bpc = len(klo)
    kblk = [a + b for a, b in zip(klo, khi)]
    tot = sum(kblk)
    nch = in_ch // P
    npc_pad = bpc * dblk
    sc_mm_dt = sc_mm_dt or sc_dt
    fin_mm_dt = fin_mm_dt or F32

    def sc_cast(ap):
        return ap

    def fin_cast(ap):
        return ap

    nc = bacc.Bacc(
        "TRN2", target_bir_lowering=False, debug=False,
        num_swdge_queues=4, dynamic_dma_scratch_size=32768,
    )
    x_ds = [
        nc.dram_tensor(f"x{h}", [half, in_ch], sc_mm_dt, kind="ExternalInput")
        for h in range(2)
    ]
    si_d = nc.dram_tensor("idx16", [P, 8 * tot], mybir.dt.int16, kind="ExternalInput")
    dl_d = nc.dram_tensor("dstl", [P, tot], F32, kind="ExternalInput")
    nm_d = nc.dram_tensor("nrm", [P, tot], F32, kind="ExternalInput")
    dln_d = nc.dram_tensor("dlneg", [P, tot], F32, kind="ExternalInput")
    nmn_d = nc.dram_tensor("nrmneg", [P, tot], F32, kind="ExternalInput")
    io_d = nc.dram_tensor("iota", [P, dblk], sc_mm_dt, kind="ExternalInput")
    xs_d = nc.dram_tensor("xself", [npc_pad, in_ch], sc_mm_dt, kind="ExternalInput")
    wt_ds = [
        nc.dram_tensor(f"wt{h}", [P, hid], fin_mm_dt, kind="ExternalInput")
        for h in range(nch)
    ]
    bs_d = nc.dram_tensor("bias", [1, hid], fin_mm_dt, kind="ExternalInput")
    on_d = nc.dram_tensor("ones", [1, P], fin_mm_dt, kind="ExternalInput")
    idr_d = nc.dram_tensor("idr", [P, P], sc_mm_dt, kind="ExternalInput")
    out_d = nc.dram_tensor("out", [npc, hid], F32, kind="ExternalOutput")

    with tile.TileContext(nc) as tc, ExitStack() as ctx:
        const = ctx.enter_context(tc.tile_pool(name="const", bufs=1))
        gx_bufs = max(2, (12 if sc_mm_dt == BF16 else 8) // GBLK)
        gxp = ctx.enter_context(tc.tile_pool(name="gx", bufs=gx_bufs))
        mselp = ctx.enter_context(tc.tile_pool(name="msel", bufs=6))
        psA = ctx.enter_context(tc.tile_pool(name="psA", bufs=2, space="PSUM"))
        psT = ctx.enter_context(tc.tile_pool(name="psT", bufs=1, space="PSUM"))
        hps = ctx.enter_context(tc.tile_pool(name="hps", bufs=3, space="PSUM"))
        aS = ctx.enter_context(tc.tile_pool(name="aS", bufs=3))
        xsp = ctx.enter_context(tc.tile_pool(name="xsp", bufs=3))
        outp = ctx.enter_context(tc.tile_pool(name="outp", bufs=6))

        si_t = const.tile([P, 8 * tot], mybir.dt.int16)
        nc.sync.dma_start(out=si_t[:], in_=si_d.ap())
        dl_t = const.tile([P, tot], F32)
        nc.sync.dma_start(out=dl_t[:], in_=dl_d.ap())
        nm_t = const.tile([P, tot], F32)
        nc.sync.dma_start(out=nm_t[:], in_=nm_d.ap())
        if MSACT > 0:
            dln_t = const.tile([P, tot], F32)
            nc.sync.dma_start(out=dln_t[:], in_=dln_d.ap())
            nmn_t = const.tile([P, tot], F32)
            nc.sync.dma_start(out=nmn_t[:], in_=nmn_d.ap())
        io_t = const.tile([P, dblk], sc_mm_dt)
        nc.sync.dma_start(out=io_t[:], in_=io_d.ap())
        wt_t = []
        for h in range(nch):
            w = const.tile([P, hid], fin_mm_dt, name=f"wt_t{h}")
            nc.sync.dma_start(out=w[:], in_=wt_ds[h].ap())
            wt_t.append(w)
        bs_t = const.tile([1, hid], fin_mm_dt)
        nc.sync.dma_start(out=bs_t[:], in_=bs_d.ap())
        on_t = const.tile([1, P], fin_mm_dt)
        nc.sync.dma_start(out=on_t[:], in_=on_d.ap())
        id_t = const.tile([P, P], F32)
        make_identity(nc, id_t[:])
        idr_t = const.tile([P, P], sc_mm_dt)
        nc.sync.dma_start(out=idr_t[:], in_=idr_d.ap())

        # group-level chunk bases (same layout as _preprocess)
        segbase = np.zeros((bpc, 2), np.int64)
        off = 0
        groups = []
        for g0 in range(0, bpc, GBLK):
            blocks = list(range(g0, min(g0 + GBLK, bpc)))
            for b in blocks:
                segbase[b, 0] = off
                off += klo[b]
            for b in blocks:
                segbase[b, 1] = off
                off += khi[b]
            groups.append(blocks)

        gather_qn = 0
        for blocks in groups:
            b0 = blocks[0]
            kg = [sum(klo[b] for b in blocks), sum(khi[b] for b in blocks)]
            gstart = [int(segbase[b0, 0]), int(segbase[b0, 1])]
            gxs = []
            for h in range(2):
                if kg[h] == 0:
                    gxs.append(None)
                    continue
                nidx = kg[h] * P
                gx = gxp.tile(
                    [P, kg[h] * in_ch], sc_mm_dt, tag=f"gx{h}", name=f"gx{h}_{b0}"
                )
                nc.gpsimd.dma_gather(
                    gx[:].rearrange("p (k d) -> p k d", d=in_ch),
                    x_ds[h].ap(),
                    si_t[:, 8 * gstart[h] : 8 * (gstart[h] + kg[h])],
                    nidx,
                    nidx,
                    in_ch,
                    queue_num=gather_qn % 4,
                    single_packet=False,
                )
                gather_qn += 1
                gxs.append(gx)
            for b in blocks:
                nb = min(dblk, npc - b * dblk)
                A = psA.tile([P, in_ch], F32, tag="A", name=f"A_{b}")
                first = True
                for h in range(2):
                    gx = gxs[h]
                    koff = int(segbase[b, h]) - gstart[h]
                    kk = (klo, khi)[h][b]
                    for j in range(kk):
                        ci = int(segbase[b, h]) + j
                        jj = koff + j
                        ms = mselp.tile(
                            [P, dblk], sc_mm_dt, tag="ms", name=f"ms_{b}_{h}_{j}"
                        )
                        if ci % 8 < MSACT:
                            # ScalarE build: ms = relu(nrm - nrm*|iota - dst|)
                            mt = mselp.tile(
                                [P, dblk], sc_mm_dt, tag="mt", name=f"mt_{b}_{h}_{j}"
                            )
                            nc.scalar.activation(
                                out=mt[:],
                                in_=io_t[:],
                                func=mybir.ActivationFunctionType.Abs,
                                bias=dln_t[:, ci : ci + 1],
                            )
                            nc.scalar.activation(
                                out=ms[:],
                                in_=mt[:],
                                func=mybir.ActivationFunctionType.Relu,
                                scale=nmn_t[:, ci : ci + 1],
                                bias=nm_t[:, ci : ci + 1],
                            )
                        else:
                            nc.vector.tensor_scalar(
                                out=ms[:],
                                in0=io_t[:],
                                scalar1=dl_t[:, ci : ci + 1],
                                scalar2=nm_t[:, ci : ci + 1],
                                op0=mybir.AluOpType.is_equal,
                                op1=mybir.AluOpType.mult,
                            )
                        nc.tensor.matmul(
                            A[:],
                            lhsT=ms[:],
                            rhs=gx[:, jj * in_ch : (jj + 1) * in_ch],
                            start=first,
                            stop=False,
                        )
                        first = False
                # A[d, c] += dinv[d]^2 * x[d, c] (host-prescaled), via identity mm
                xs_t = xsp.tile([P, in_ch], sc_mm_dt, tag="xs", name=f"xs_{b}")
                nc.sync.dma_start(
                    out=xs_t[:], in_=xs_d.ap()[b * dblk : (b + 1) * dblk, :]
                )
                nc.tensor.matmul(
                    A[:], lhsT=idr_t[:], rhs=xs_t[:], start=first, stop=True
                )
                a_s = aS.tile([P, in_ch], F32, tag="as", name=f"as_{b}")
                nc.scalar.copy(a_s[:], A[:])
                # transpose A halves on the PE: AT[c, d] = A[d, c]^T
                at_s = []
                for h in range(nch):
                    atp = psT.tile([P, P], F32, tag=f"atp{h}", name=f"atp{h}_{b}")
                    nc.tensor.transpose(
                        out=atp[:], in_=a_s[:, h * P : (h + 1) * P], identity=id_t[:]
                    )
                    ats = aS.tile([P, P], fin_mm_dt, tag=f"ats{h}", name=f"ats{h}_{b}")
                    nc.scalar.copy(ats[:], atp[:])
                    at_s.append(ats)
                ns = nb
                Hp = hps.tile([P, hid], F32, tag="hp", name=f"hp_{b}")
                for h in range(nch):
                    nc.tensor.matmul(
                        Hp[:ns],
                        lhsT=fin_cast(at_s[h][:, :ns]),
                        rhs=fin_cast(wt_t[h][:]),
                        start=(h == 0),
                        stop=False,
                    )
                nc.tensor.matmul(
                    Hp[:ns],
                    lhsT=fin_cast(on_t[:, :ns]),
                    rhs=fin_cast(bs_t[:]),
                    start=False,
                    stop=True,
                )
                os_ = outp.tile([P, hid], F32, tag="os", name=f"os_{b}")
                t2 = outp.tile([P, hid], F32, tag="t2", name=f"t2_{b}")
                if 0.0 <= alpha <= 1.0:
                    # PReLU = max(H, alpha*H)
                    nc.scalar.activation(
                        out=t2[:ns],
                        in_=Hp[:ns],
                        func=mybir.ActivationFunctionType.Copy,
                        scale=float(alpha),
                    )
                    nc.vector.tensor_tensor(
                        out=os_[:ns], in0=t2[:ns], in1=Hp[:ns], op=mybir.AluOpType.max
                    )
                else:
                    # general PReLU: relu(H)*(1-alpha) + alpha*H
                    nc.scalar.activation(
                        out=t2[:ns],
                        in_=Hp[:ns],
                        func=mybir.ActivationFunctionType.Relu,
                    )
                    nc.vector.tensor_scalar(
                        out=t2[:ns],
                        in0=t2[:ns],
                        scalar1=float(1.0 - alpha),
                        scalar2=None,
                        op0=mybir.AluOpType.mult,
                    )
                    t3 = outp.tile([P, hid], F32, tag="t3", name=f"t3_{b}")
                    nc.vector.tensor_scalar(
                        out=t3[:ns],
                        in0=Hp[:ns],
                        scalar1=float(alpha),
                        scalar2=None,
                        op0=mybir.AluOpType.mult,
                    )
                    nc.vector.tensor_tensor(
                        out=os_[:ns], in0=t2[:ns], in1=t3[:ns], op=mybir.AluOpType.add
                    )
                row0 = b * dblk
                nc.sync.dma_start(
                    out=out_d.ap()[row0 : row0 + ns, :], in_=os_[:ns, :]
                )
    nc.compile()
    return nc


def _make_in_maps(
    x, weight, bias, idx16, dstl, nrm, dinv, sc_np=np.float32, ncores=NCORES
):
    x = np.asarray(x, dtype=np.float32)
    w = np.asarray(weight, dtype=np.float32)
    n = x.shape[0]
    half = n // 2
    in_ch = x.shape[1]
    hid = w.shape[0]
    npc = n // ncores
    bpc = (npc + P - 1) // P
    npc_pad = bpc * P
    iota = np.tile(np.arange(P, dtype=sc_np), (P, 1))
    wts = {
        f"wt{h}": np.ascontiguousarray(w[:, h * P : (h + 1) * P].T)
        for h in range(in_ch // P)
    }
    bias_row = np.asarray(bias, dtype=np.float32).reshape(1, hid)
    xlo = np.ascontiguousarray(x[:half].astype(sc_np))
    xhi = np.ascontiguousarray(x[half:].astype(sc_np))
    xself_all = x * (dinv * dinv)[:, None]  # [n, in_ch] f32
    in_maps = []
    for k in range(ncores):
        xs = np.zeros((npc_pad, in_ch), sc_np)
        xs[:npc] = xself_all[k * npc : (k + 1) * npc].astype(sc_np)
        m = {
            "x0": xlo,
            "x1": xhi,
            "idx16": np.ascontiguousarray(idx16[k]),
            "dstl": np.ascontiguousarray(dstl[k]),
            "nrm": np.ascontiguousarray(nrm[k]),
            "dlneg": np.ascontiguousarray(-dstl[k]),
            "nrmneg": np.ascontiguousarray(-nrm[k]),
            "iota": iota,
            "xself": xs,
            "bias": bias_row,
            "ones": np.ones((1, P), np.float32),
            "idr": np.eye(P, dtype=sc_np),
        }
        m.update(wts)
        in_maps.append(m)
    return in_maps


# Results of the last kernel() call, for the test harness.
LAST_RESULTS = None


def _dt_opts():
    sc = os.environ.get("GCN_SC_DT", "f32r")
    fin = os.environ.get("GCN_FIN_DT", "f32r")
    sc_dt = {"f32": F32, "f32r": F32, "bf16": BF16}[sc]
    sc_mm_dt = {"f32": F32, "f32r": mybir.dt.float32r, "bf16": BF16}[sc]
    fin_mm_dt = {"f32": F32, "f32r": mybir.dt.float32r}[fin]
    sc_np = np.float32 if sc_dt == F32 else mybir.dt.np(BF16)
    return sc_dt, sc_mm_dt, fin_mm_dt, sc_np


def kernel(x, edge_index, weight, bias, prelu_a):
    global LAST_RESULTS
    sc_dt, sc_mm_dt, fin_mm_dt, sc_np = _dt_opts()
    trace = os.environ.get("GCN_TRACE", "0") == "1"

    klo, khi, idx16, dstl, nrm, dinv = _preprocess(edge_index)
    alpha = float(np.asarray(prelu_a).ravel()[0])
    nc = _build_program(
        klo, khi, alpha, sc_dt=sc_dt, sc_mm_dt=sc_mm_dt, fin_mm_dt=fin_mm_dt
    )
    in_maps = _make_in_maps(x, weight, bias, idx16, dstl, nrm, dinv, sc_np=sc_np)

    res = bass_utils.run_bass_kernel_spmd(
        nc, in_maps, core_ids=list(range(NCORES)), trace=trace
    )
    LAST_RESULTS = res
    out = np.concatenate([res.results[k]["out"] for k in range(NCORES)], axis=0)
    return out.astype(np.float32)



# revision 10
# speedup vs baseline: 1.0040x; 1.0040x over previous
"""GCN encoder (GCNConv + PReLU) as a Bass/Tile kernel on 8 Trainium2 NeuronCores.

Math (matches PyG GCNConv with self-loops + symmetric norm, then PReLU):
    deg[i]  = in-degree of i over dst (+1 self loop)
    dinv    = 1/sqrt(deg)
    agg[d]  = sum_{e:(s->d)} dinv[s]*dinv[d] * x[s] + dinv[d]^2 * x[d]
    out     = PReLU(agg @ W.T + bias)

Distribution: dst-node sharding, core k owns nodes [k*6250, (k+1)*6250).

Key structure (v2):
  - x is host-prescaled by dinv[src] and stored bf16 in two 25000-row halves
    (int16 gather indices). Per dst-group (GBLK blocks of 128 dst), edges are
    packed contiguously (block-major) and fetched with dma_gather; trailing
    index slots are -1 so the Q7 descriptor generator trims them (desc-gen
    cost == real edge count, no cross-core padding cost).
  - Msel[e, d] = (iota == dstl[e]) is a pure 0/1 selection built with a single
    DVE is_equal op (bf16), optionally some on ACT via relu(1 - |iota-dstl|).
  - The scatter-add accumulates the TRANSPOSED aggregate directly:
        AT_h[c, d] += gx[e, c]^T @ Msel[e, d]   (PE, bf16, PSUM f32)
    so no PE transpose pass is needed before the weight matmul.
  - Self-loop: AT_h += xsall[d, c]^T @ I via one identity matmul per half
    (xsall = dinv * x own rows, dense HWDGE load).
  - H[d, :] = sum_h AT_h^T @ W_h + sqrtdeg[d] * bias  (PSUM accumulation;
    the sqrtdeg row makes the bias exact after the final dinv[d] scaling).
  - out = Prelu(H * dinv[d]) in ONE scalar-engine activation (parametric_relu
    is resident in every activation table; alpha is the PReLU scalar).

Env knobs: GCN_GBLK (2), GCN_NBUF (8), GCN_MSACT (0..8: of every 8 msel
builds, this many go to ACT), GCN_PRELU=act|max.
"""

import os
import numpy as np

import concourse.bass as bass
import concourse.tile as tile
from concourse import bacc, mybir, bass_utils
from contextlib import ExitStack

# Problem shape (fixed by the harness contract).
N_NODES = 50000
N_EDGES = 400000
IN_CH = 256
HID = 512
NCORES = 8
NPC = N_NODES // NCORES  # dst nodes owned per core
P = 128
BPC = (NPC + P - 1) // P  # dst blocks per core (49)

F32 = mybir.dt.float32
F32R = mybir.dt.float32r
BF16 = mybir.dt.bfloat16

GBLK = int(os.environ.get("GCN_GBLK", "2"))
FULLGEN = os.environ.get("GCN_FULLGEN", "0") == "1"
NBUF = int(os.environ.get("GCN_NBUF", "8"))
MSACT = int(os.environ.get("GCN_MSACT", "0"))
PRELU_MODE = os.environ.get("GCN_PRELU", "act")


def _preprocess(edge_index, n_nodes=N_NODES, ncores=NCORES, gblk=GBLK, nbuf=NBUF):
    """Group non-self edges by (core, dst-group, src-half), pack block-major.

    Returns (sched, kmax, npairs, idx16, dstl, cnts, dinv):
      sched: per group g: dict(kk=[Klo,Khi] static chunk counts,
             nidx=[...] static gather num_idxs,
             mm={h: [(j, local_b, paircol), ...]} static matmul schedule)
      idx16: [ncores, 128, 2*ngroups*8*kmax] int16 gather indices
      dstl:  [ncores, 128, npairs] f32; dst-local in block or -1
      dinv:  [n_nodes] f32
    """
    npc = n_nodes // ncores
    half = n_nodes // 2
    ngroups = (BPC + gblk - 1) // gblk
    src = np.asarray(edge_index[0]).astype(np.int64).ravel()
    dst = np.asarray(edge_index[1]).astype(np.int64).ravel()
    deg = np.bincount(dst, minlength=n_nodes).astype(np.float32) + 1.0
    dinv = (1.0 / np.sqrt(deg)).astype(np.float32)

    core = dst // npc
    dloc = dst - core * npc
    blk = dloc // P
    grp = blk // gblk
    hi = (src >= half).astype(np.int64)

    # sort edges by (core, grp, hi, blk, src)
    key = (((core * ngroups + grp) * 2 + hi) * BPC + blk) * (half + 1) + (
        src - hi * half
    )
    order = np.argsort(key, kind="stable")
    src_s, core_s = src[order], core[order]
    grp_s, hi_s, blk_s = grp[order], hi[order], blk[order]
    dll_s = (dloc[order] - blk_s * P).astype(np.int64)  # dst local in block

    # counts per (core, grp, hi) and per (core, grp, hi, blk)
    cgh = (core_s * ngroups + grp_s) * 2 + hi_s
    n_cgh = ncores * ngroups * 2
    cnt_cgh = np.bincount(cgh, minlength=n_cgh).reshape(ncores, ngroups, 2)
    cghb = cgh * BPC + blk_s
    cnt_cghb = np.bincount(cghb, minlength=n_cgh * BPC).reshape(
        ncores, ngroups, 2, BPC
    )

    kgh = -(-cnt_cgh.max(axis=0) // P)  # [ngroups, 2] static chunk counts
    kgh = np.maximum(kgh, 1)
    kmax = int(kgh.max())

    # rank of each edge within its (core, grp, hi) segment
    seg_start = np.zeros(n_cgh + 1, np.int64)
    seg_start[1:] = np.cumsum(cnt_cgh.ravel())
    rank = np.arange(len(order)) - seg_start[cgh]

    # static matmul schedule: union over cores of blocks present in chunk j
    # block b of (g,h) occupies ranks [bs, bs+cnt) -> chunks bs//P .. (bs+cnt-1)//P
    bstart = np.cumsum(cnt_cghb, axis=3) - cnt_cghb  # [nc, ng, 2, BPC]
    sched = []
    npairs = 0
    paircol = {}
    for g in range(ngroups):
        blocks = list(range(g * gblk, min((g + 1) * gblk, BPC)))
        mm = {0: [], 1: []}
        for h in range(2):
            k = int(kgh[g, h])
            for j in range(k):
                for b in blocks:
                    bl = b - g * gblk
                    lo = bstart[:, g, h, b]
                    cnt = cnt_cghb[:, g, h, b]
                    # does block b intersect chunk j on any core?
                    inter = np.any(
                        (cnt > 0) & (lo < (j + 1) * P) & (lo + cnt > j * P)
                    )
                    if inter:
                        paircol[(g, h, j, b)] = npairs
                        mm[h].append((j, bl, npairs))
                        npairs += 1
        nidx = [
            (kmax if g < nbuf else int(kgh[g, h])) * P for h in range(2)
        ]
        sched.append(
            {"kk": [int(kgh[g, 0]), int(kgh[g, 1])], "nidx": nidx, "mm": mm}
        )

    # per-core data arrays
    segw = 8 * kmax  # idx16 columns per (g,h) segment
    fill = 0 if FULLGEN else -1
    idx16 = np.full((ncores, 16, 2 * ngroups * segw), fill, np.int16)
    # first-rotation groups: pad with 0 (valid) so gx pool buffers are
    # fully written once before trimmed gathers leave stale tails
    for g in range(min(nbuf, ngroups)):
        for h in range(2):
            s = (g * 2 + h) * segw
            idx16[:, :, s : s + segw] = 0
    dstl = np.full((ncores, P, npairs), -1.0, np.float32)

    seg = (grp_s * 2 + hi_s) * segw
    col = seg + (rank // 16)
    row = rank % 16
    idx16[core_s, row, col] = (src_s - hi_s * half).astype(np.int16)

    # dstl entries: edge at rank r -> chunk j=r//P, slot p=r%P; pair (g,h,j,blk)
    j_s = rank // P
    p_s = rank % P
    pc = np.array(
        [paircol[(g, h, j, b)] for g, h, j, b in zip(grp_s, hi_s, j_s, blk_s)],
        dtype=np.int64,
    )
    dstl[core_s, p_s, pc] = dll_s.astype(np.float32)

    idx16 = np.tile(idx16, (1, 8, 1))  # replicate to 128 partitions

    # runtime gather counts per (core, g, h): real edges (trailing -1 slots
    # are trimmed by the Q7 desc generator); first-rotation groups gather
    # their full static extent (padded with index 0)
    cnts = np.empty((ncores, 2 * ngroups), np.int32)
    for g in range(ngroups):
        for h in range(2):
            if g < nbuf:
                cnts[:, g * 2 + h] = kmax * P
            elif FULLGEN:
                cnts[:, g * 2 + h] = int(
                    np.ceil(cnt_cgh[:, g, h].max() / P)
                ) * P
            else:
                cnts[:, g * 2 + h] = cnt_cgh[:, g, h]
    return sched, kmax, npairs, idx16, dstl, cnts, dinv


def _build_program(
    sched,
    kmax,
    npairs,
    alpha,
    n_nodes=N_NODES,
    ncores=NCORES,
    in_ch=IN_CH,
    hid=HID,
    gblk=GBLK,
    nbuf=NBUF,
    msact=MSACT,
    prelu_mode=PRELU_MODE,
):
    npc = n_nodes // ncores
    ngroups = len(sched)
    segw = 8 * kmax
    nch = in_ch // P  # 2 channel halves

    nc = bacc.Bacc(
        "TRN2", target_bir_lowering=False, debug=False,
        num_swdge_queues=4, dynamic_dma_scratch_size=32768,
    )
    half = n_nodes // 2
    x_ds = [
        nc.dram_tensor(f"x{h}", [half, in_ch], BF16, kind="ExternalInput")
        for h in range(2)
    ]
    si_d = nc.dram_tensor(
        "idx16", [P, 2 * ngroups * segw], mybir.dt.int16, kind="ExternalInput"
    )
    dl_d = nc.dram_tensor("dstl", [P, npairs], F32, kind="ExternalInput")
    dln_d = nc.dram_tensor("dlneg", [P, npairs], F32, kind="ExternalInput")
    io_d = nc.dram_tensor("iota", [P, P], BF16, kind="ExternalInput")
    xs_d = nc.dram_tensor("xsall", [P, BPC * in_ch], BF16, kind="ExternalInput")
    wt_ds = [
        nc.dram_tensor(f"wt{h}", [P, hid], BF16, kind="ExternalInput")
        for h in range(nch)
    ]
    bs_d = nc.dram_tensor("bias", [1, hid], F32R, kind="ExternalInput")
    sdg_d = nc.dram_tensor("sdg", [1, BPC * P], F32R, kind="ExternalInput")
    idr_d = nc.dram_tensor("idr", [P, P], BF16, kind="ExternalInput")
    ct_d = nc.dram_tensor(
        "cnts", [1, 2 * ngroups], mybir.dt.int32, kind="ExternalInput"
    )
    dv_d = nc.dram_tensor("dinvc", [P, BPC], F32, kind="ExternalInput")
    adv_d = nc.dram_tensor("adinvc", [P, BPC], F32, kind="ExternalInput")
    out_d = nc.dram_tensor("out", [npc, hid], F32, kind="ExternalOutput")

    with tile.TileContext(nc) as tc, ExitStack() as ctx:
        const = ctx.enter_context(tc.tile_pool(name="const", bufs=1))
        gxp = ctx.enter_context(tc.tile_pool(name="gx", bufs=nbuf))
        mselp = ctx.enter_context(tc.tile_pool(name="msel", bufs=8))
        psA = ctx.enter_context(tc.tile_pool(name="psA", bufs=2, space="PSUM"))
        hps = ctx.enter_context(tc.tile_pool(name="hps", bufs=3, space="PSUM"))
        aS = ctx.enter_context(tc.tile_pool(name="aS", bufs=4))
        outp = ctx.enter_context(tc.tile_pool(name="outp", bufs=4))

        si_t = const.tile([P, 2 * ngroups * segw], mybir.dt.int16)
        nc.sync.dma_start(out=si_t[:], in_=si_d.ap())
        dl_t = const.tile([P, npairs], F32)
        nc.sync.dma_start(out=dl_t[:], in_=dl_d.ap())
        if msact > 0:
            dln_t = const.tile([P, npairs], F32)
            nc.sync.dma_start(out=dln_t[:], in_=dln_d.ap())
        io_t = const.tile([P, P], BF16)
        nc.sync.dma_start(out=io_t[:], in_=io_d.ap())
        xs_t = const.tile([P, BPC * in_ch], BF16)
        nc.sync.dma_start(out=xs_t[:], in_=xs_d.ap())
        wt_t = []
        for h in range(nch):
            w = const.tile([P, hid], BF16, name=f"wt_t{h}")
            nc.sync.dma_start(out=w[:], in_=wt_ds[h].ap())
            wt_t.append(w)
        bs_t = const.tile([1, hid], F32R)
        nc.sync.dma_start(out=bs_t[:], in_=bs_d.ap())
        sdg_t = const.tile([1, BPC * P], F32R)
        nc.sync.dma_start(out=sdg_t[:], in_=sdg_d.ap())
        idr_t = const.tile([P, P], BF16)
        nc.sync.dma_start(out=idr_t[:], in_=idr_d.ap())
        ct_t = const.tile([1, 2 * ngroups], mybir.dt.int32)
        nc.sync.dma_start(out=ct_t[:], in_=ct_d.ap())
        greg = nc.alloc_register(mybir.EngineType.Pool, "gcnt")
        dv_t = const.tile([P, BPC], F32)
        nc.sync.dma_start(out=dv_t[:], in_=dv_d.ap())
        adv_t = const.tile([P, BPC], F32)
        nc.sync.dma_start(out=adv_t[:], in_=adv_d.ap())

        gather_qn = 0
        paircnt = 0
        for g in range(ngroups):
            sg = sched[g]
            blocks = list(range(g * gblk, min((g + 1) * gblk, BPC)))
            nbl = len(blocks)
            # gathers (lo/hi)
            gxs = []
            for h in range(2):
                nidx = sg["nidx"][h]
                gx = gxp.tile(
                    [P, kmax * in_ch], BF16, tag=f"gx{h}", name=f"gx{h}_{g}"
                )
                kk = nidx // P
                soff = (g * 2 + h) * segw
                nc.gpsimd.reg_load(greg, ct_t[0:1, g * 2 + h : g * 2 + h + 1])
                nc.gpsimd.dma_gather(
                    gx[:, : kk * in_ch].rearrange("p (k d) -> p k d", d=in_ch),
                    x_ds[h].ap(),
                    si_t[:, soff : soff + 8 * kk],
                    nidx,
                    greg,
                    in_ch,
                    queue_num=gather_qn % 4,
                    single_packet=False,
                )
                gather_qn += 1
                gxs.append(gx)
            # AT accumulators per (local block, ch half)
            AT = {}
            started = {}
            for bl in range(nbl):
                t = psA.tile([P, nch * P], F32, tag=f"at{bl}", name=f"at{bl}_{g}")
                for hh in range(nch):
                    AT[(bl, hh)] = t[:, hh * P : (hh + 1) * P]
                started[bl] = False
            # chunk matmuls
            for h in range(2):
                for (j, bl, col) in sg["mm"][h]:
                    ms = mselp.tile(
                        [P, P], BF16, tag="ms", name=f"ms_{g}_{h}_{j}_{bl}"
                    )
                    if paircnt % 8 < msact:
                        mt = mselp.tile(
                            [P, P], BF16, tag="mt", name=f"mt_{g}_{h}_{j}_{bl}"
                        )
                        nc.scalar.activation(
                            out=mt[:],
                            in_=io_t[:],
                            func=mybir.ActivationFunctionType.Abs,
                            bias=dln_t[:, col : col + 1],
                        )
                        nc.scalar.activation(
                            out=ms[:],
                            in_=mt[:],
                            func=mybir.ActivationFunctionType.Relu,
                            scale=-1.0,
                            bias=1.0,
                        )
                    else:
                        nc.vector.tensor_scalar(
                            out=ms[:],
                            in0=io_t[:],
                            scalar1=dl_t[:, col : col + 1],
                            scalar2=None,
                            op0=mybir.AluOpType.is_equal,
                        )
                    paircnt += 1
                    gx = gxs[h]
                    for hh in range(nch):
                        nc.tensor.matmul(
                            AT[(bl, hh)],
                            lhsT=gx[:, j * in_ch + hh * P : j * in_ch + (hh + 1) * P],
                            rhs=ms[:],
                            start=not started[bl],
                            stop=False,
                        )
                        started[bl] = True
            # per block: self-loop (stop), AT->SBUF, weight mm, PReLU, store
            for bl, b in enumerate(blocks):
                ns = min(P, npc - b * P)
                ats = []
                for hh in range(nch):
                    nc.tensor.matmul(
                        AT[(bl, hh)],
                        lhsT=xs_t[:, b * in_ch + hh * P : b * in_ch + (hh + 1) * P],
                        rhs=idr_t[:],
                        start=not started[bl],
                        stop=(hh == nch - 1),
                    )
                    started[bl] = True
                for hh in range(nch):
                    a = aS.tile([P, P], BF16, tag=f"ats{hh}", name=f"ats{hh}_{b}")
                    nc.scalar.copy(a[:], AT[(bl, hh)])
                    ats.append(a)
                Hp = hps.tile([P, hid], F32, tag="hp", name=f"hp_{b}")
                for hh in range(nch):
                    nc.tensor.matmul(
                        Hp[:ns],
                        lhsT=ats[hh][:, :ns],
                        rhs=wt_t[hh][:],
                        start=(hh == 0),
                        stop=False,
                    )
                nc.tensor.matmul(
                    Hp[:ns],
                    lhsT=sdg_t[:, b * P : b * P + ns],
                    rhs=bs_t[:],
                    start=False,
                    stop=True,
                )
                os_ = outp.tile([P, hid], F32, tag="os", name=f"os_{b}")
                if prelu_mode == "act":
                    # PReLU(dinv*H) in one ACT op (exact for any alpha)
                    nc.scalar.activation(
                        out=os_[:ns],
                        in_=Hp[:ns],
                        func=mybir.ActivationFunctionType.Prelu,
                        scale=dv_t[:ns, b : b + 1],
                        alpha=float(alpha),
                    )
                else:
                    # fallback: max(dinv*H, alpha*dinv*H) (0<=alpha<=1)
                    t1 = outp.tile([P, hid], F32, tag="t1", name=f"t1_{b}")
                    t2 = outp.tile([P, hid], F32, tag="t2", name=f"t2_{b}")
                    nc.scalar.activation(
                        out=t1[:ns],
                        in_=Hp[:ns],
                        func=mybir.ActivationFunctionType.Copy,
                        scale=dv_t[:ns, b : b + 1],
                    )
                    nc.scalar.activation(
                        out=t2[:ns],
                        in_=Hp[:ns],
                        func=mybir.ActivationFunctionType.Copy,
                        scale=adv_t[:ns, b : b + 1],
                    )
                    nc.vector.tensor_tensor(
                        out=os_[:ns], in0=t1[:ns], in1=t2[:ns],
                        op=mybir.AluOpType.max,
                    )
                row0 = b * P
                nc.sync.dma_start(
                    out=out_d.ap()[row0 : row0 + ns, :], in_=os_[:ns, :]
                )
    nc.compile()
    return nc


def _make_in_maps(x, weight, bias, idx16, dstl, cnts, dinv, alpha, ncores=NCORES):
    x = np.asarray(x, dtype=np.float32)
    w = np.asarray(weight, dtype=np.float32)
    n = x.shape[0]
    half = n // 2
    in_ch = x.shape[1]
    hid = w.shape[0]
    npc = n // ncores
    bf = mybir.dt.np(BF16)

    xp = x * dinv[:, None]  # prescaled by dinv[src]
    xlo = np.ascontiguousarray(xp[:half].astype(bf))
    xhi = np.ascontiguousarray(xp[half:].astype(bf))
    iota = np.tile(np.arange(P, dtype=bf), (P, 1))
    wts = {
        f"wt{h}": np.ascontiguousarray(w[:, h * P : (h + 1) * P].T.astype(bf))
        for h in range(in_ch // P)
    }
    bias_row = np.asarray(bias, dtype=np.float32).reshape(1, hid)
    sdeg = (1.0 / dinv).astype(np.float32)  # sqrt(deg)

    in_maps = []
    for k in range(ncores):
        sl = slice(k * npc, (k + 1) * npc)
        # xsall[d, b*256 + c] = (dinv*x)[k*npc + b*128 + d, c]
        xs = np.zeros((BPC * P, in_ch), np.float32)
        xs[:npc] = xp[sl]
        xsall = np.ascontiguousarray(
            xs.reshape(BPC, P, in_ch).transpose(1, 0, 2).reshape(P, BPC * in_ch)
        ).astype(bf)
        dv = np.zeros((BPC * P,), np.float32)
        dv[:npc] = dinv[sl]
        dvc = np.ascontiguousarray(dv.reshape(BPC, P).T)
        sdg = np.zeros((1, BPC * P), np.float32)
        sdg[0, :npc] = sdeg[sl]
        m = {
            "x0": xlo,
            "x1": xhi,
            "idx16": np.ascontiguousarray(idx16[k]),
            "cnts": np.ascontiguousarray(cnts[k : k + 1]),
            "dstl": np.ascontiguousarray(dstl[k]),
            "dlneg": np.ascontiguousarray(-dstl[k]),
            "iota": iota,
            "xsall": xsall,
            "bias": bias_row,
            "sdg": sdg,
            "idr": np.eye(P, dtype=bf),
            "dinvc": dvc,
            "adinvc": np.ascontiguousarray(dvc * float(alpha)),
        }
        m.update(wts)
        in_maps.append(m)
    return in_maps


# Results of the last kernel() call, for the test harness.
LAST_RESULTS = None


def kernel(x, edge_index, weight, bias, prelu_a):
    global LAST_RESULTS
    trace = os.environ.get("GCN_TRACE", "0") == "1"

    sched, kmax, npairs, idx16, dstl, cnts, dinv = _preprocess(edge_index)
    alpha = float(np.asarray(prelu_a).ravel()[0])
    nc = _build_program(sched, kmax, npairs, alpha)
    in_maps = _make_in_maps(x, weight, bias, idx16, dstl, cnts, dinv, alpha)

    res = bass_utils.run_bass_kernel_spmd(
        nc, in_maps, core_ids=list(range(NCORES)), trace=trace
    )
    LAST_RESULTS = res
    out = np.concatenate([res.results[k]["out"] for k in range(NCORES)], axis=0)
    return out.astype(np.float32)


# revision 14
# speedup vs baseline: 1.2655x; 1.2605x over previous
"""GCN encoder (GCNConv + PReLU) as a Bass/Tile kernel on 8 Trainium2 NeuronCores.

Math (matches PyG GCNConv with self-loops + symmetric norm, then PReLU):
    deg[i]  = in-degree of i over dst (+1 self loop)
    dinv    = 1/sqrt(deg)
    agg[d]  = sum_{e:(s->d)} dinv[s]*dinv[d] * x[s] + dinv[d]^2 * x[d]
    out     = PReLU(agg @ W.T + bias)

Distribution: dst-node sharding, core k owns nodes [k*6250, (k+1)*6250).

Key structure (v2):
  - x is host-prescaled by dinv[src] and stored bf16 in two 25000-row halves
    (int16 gather indices). Per dst-group (GBLK blocks of 128 dst), edges are
    packed contiguously (block-major) and fetched with dma_gather; trailing
    index slots are -1 so the Q7 descriptor generator trims them (desc-gen
    cost == real edge count, no cross-core padding cost).
  - Msel[e, d] = (iota == dstl[e]) is a pure 0/1 selection built with a single
    DVE is_equal op (bf16), optionally some on ACT via relu(1 - |iota-dstl|).
  - The scatter-add accumulates the TRANSPOSED aggregate directly:
        AT_h[c, d] += gx[e, c]^T @ Msel[e, d]   (PE, bf16, PSUM f32)
    so no PE transpose pass is needed before the weight matmul.
  - Self-loop: AT_h += xsall[d, c]^T @ I via one identity matmul per half
    (xsall = dinv * x own rows, dense HWDGE load).
  - H[d, :] = sum_h AT_h^T @ W_h + sqrtdeg[d] * bias  (PSUM accumulation;
    the sqrtdeg row makes the bias exact after the final dinv[d] scaling).
  - out = Prelu(H * dinv[d]) in ONE scalar-engine activation (parametric_relu
    is resident in every activation table; alpha is the PReLU scalar).

Env knobs: GCN_GBLK (2), GCN_NBUF (8), GCN_MSACT (0..8: of every 8 msel
builds, this many go to ACT), GCN_PRELU=act|max.
"""

import os
import numpy as np

import concourse.bass as bass
import concourse.tile as tile
from concourse import bacc, mybir, bass_utils
from contextlib import ExitStack

# Problem shape (fixed by the harness contract).
N_NODES = 50000
N_EDGES = 400000
IN_CH = 256
HID = 512
NCORES = 8
NPC = N_NODES // NCORES  # dst nodes owned per core
P = 128
BPC = (NPC + P - 1) // P  # dst blocks per core (49)

F32 = mybir.dt.float32
F32R = mybir.dt.float32r
BF16 = mybir.dt.bfloat16

GBLK = int(os.environ.get("GCN_GBLK", "2"))
FULLGEN = os.environ.get("GCN_FULLGEN", "0") == "1"
NBUF = int(os.environ.get("GCN_NBUF", "8"))
MSACT = int(os.environ.get("GCN_MSACT", "0"))
PRELU_MODE = os.environ.get("GCN_PRELU", "act")


def _preprocess(edge_index, n_nodes=N_NODES, ncores=NCORES, gblk=GBLK, nbuf=NBUF):
    """Group non-self edges by (core, dst-group, src-half), pack block-major.

    Returns (sched, kmax, npairs, idx16, dstl, cnts, dinv):
      sched: per group g: dict(kk=[Klo,Khi] static chunk counts,
             nidx=[...] static gather num_idxs,
             mm={h: [(j, local_b, paircol), ...]} static matmul schedule)
      idx16: [ncores, 128, 2*ngroups*8*kmax] int16 gather indices
      dstl:  [ncores, 128, npairs] f32; dst-local in block or -1
      dinv:  [n_nodes] f32
    """
    npc = n_nodes // ncores
    half = n_nodes // 2
    ngroups = (BPC + gblk - 1) // gblk
    src = np.asarray(edge_index[0]).astype(np.int64).ravel()
    dst = np.asarray(edge_index[1]).astype(np.int64).ravel()
    deg = np.bincount(dst, minlength=n_nodes).astype(np.float32) + 1.0
    dinv = (1.0 / np.sqrt(deg)).astype(np.float32)

    core = dst // npc
    dloc = dst - core * npc
    blk = dloc // P
    grp = blk // gblk
    hi = (src >= half).astype(np.int64)

    # sort edges by (core, grp, hi, blk, src)
    key = (((core * ngroups + grp) * 2 + hi) * BPC + blk) * (half + 1) + (
        src - hi * half
    )
    order = np.argsort(key, kind="stable")
    src_s, core_s = src[order], core[order]
    grp_s, hi_s, blk_s = grp[order], hi[order], blk[order]
    dll_s = (dloc[order] - blk_s * P).astype(np.int64)  # dst local in block

    # counts per (core, grp, hi) and per (core, grp, hi, blk)
    cgh = (core_s * ngroups + grp_s) * 2 + hi_s
    n_cgh = ncores * ngroups * 2
    cnt_cgh = np.bincount(cgh, minlength=n_cgh).reshape(ncores, ngroups, 2)
    cghb = cgh * BPC + blk_s
    cnt_cghb = np.bincount(cghb, minlength=n_cgh * BPC).reshape(
        ncores, ngroups, 2, BPC
    )

    kgh = -(-cnt_cgh.max(axis=0) // P)  # [ngroups, 2] static chunk counts
    kgh = np.maximum(kgh, 1)
    kmax = int(kgh.max())

    # rank of each edge within its (core, grp, hi) segment
    seg_start = np.zeros(n_cgh + 1, np.int64)
    seg_start[1:] = np.cumsum(cnt_cgh.ravel())
    rank = np.arange(len(order)) - seg_start[cgh]

    # static matmul schedule: union over cores of blocks present in chunk j
    # block b of (g,h) occupies ranks [bs, bs+cnt) -> chunks bs//P .. (bs+cnt-1)//P
    bstart = np.cumsum(cnt_cghb, axis=3) - cnt_cghb  # [nc, ng, 2, BPC]
    sched = []
    npairs = 0
    paircol = {}
    for g in range(ngroups):
        blocks = list(range(g * gblk, min((g + 1) * gblk, BPC)))
        mm = {0: [], 1: []}
        for h in range(2):
            k = int(kgh[g, h])
            for j in range(k):
                for b in blocks:
                    bl = b - g * gblk
                    lo = bstart[:, g, h, b]
                    cnt = cnt_cghb[:, g, h, b]
                    # does block b intersect chunk j on any core?
                    inter = np.any(
                        (cnt > 0) & (lo < (j + 1) * P) & (lo + cnt > j * P)
                    )
                    if inter:
                        paircol[(g, h, j, b)] = npairs
                        mm[h].append((j, bl, npairs))
                        npairs += 1
        nidx = [
            (kmax if g < nbuf else int(kgh[g, h])) * P for h in range(2)
        ]
        pc1 = npairs
        sched.append(
            {"kk": [int(kgh[g, 0]), int(kgh[g, 1])], "nidx": nidx, "mm": mm,
             "pc0": pc1 - len(mm[0]) - len(mm[1]), "pc1": pc1}
        )

    # per-core data arrays
    segw = 8 * kmax  # idx16 columns per (g,h) segment
    fill = 0 if FULLGEN else -1
    idx16 = np.full((ncores, 16, 2 * ngroups * segw), fill, np.int16)
    # first-rotation groups: pad with 0 (valid) so gx pool buffers are
    # fully written once before trimmed gathers leave stale tails
    for g in range(min(nbuf, ngroups)):
        for h in range(2):
            s = (g * 2 + h) * segw
            idx16[:, :, s : s + segw] = 0
    seg = (grp_s * 2 + hi_s) * segw
    col = seg + (rank // 16)
    row = rank % 16
    idx16[core_s, row, col] = (src_s - hi_s * half).astype(np.int16)

    # msel one-hot tiles: edge at rank r -> chunk j=r//P, slot p=r%P;
    # pair (g,h,j,blk) -> column pc*P + dst_local
    j_s = rank // P
    p_s = rank % P
    pc = np.array(
        [paircol[(g, h, j, b)] for g, h, j, b in zip(grp_s, hi_s, j_s, blk_s)],
        dtype=np.int64,
    )
    msel = np.zeros((ncores, P, npairs * P), np.float32)
    msel[core_s, p_s, pc * P + dll_s] = 1.0

    idx16 = np.tile(idx16, (1, 8, 1))  # replicate to 128 partitions

    # runtime gather counts per (core, g, h): real edges (trailing -1 slots
    # are trimmed by the Q7 desc generator); first-rotation groups gather
    # their full static extent (padded with index 0)
    cnts = np.empty((ncores, 2 * ngroups), np.int32)
    for g in range(ngroups):
        for h in range(2):
            if g < nbuf:
                cnts[:, g * 2 + h] = kmax * P
            elif FULLGEN:
                cnts[:, g * 2 + h] = int(
                    np.ceil(cnt_cgh[:, g, h].max() / P)
                ) * P
            else:
                cnts[:, g * 2 + h] = cnt_cgh[:, g, h]
    return sched, kmax, npairs, idx16, msel, cnts, dinv


def _build_program(
    sched,
    kmax,
    npairs,
    alpha,
    n_nodes=N_NODES,
    ncores=NCORES,
    in_ch=IN_CH,
    hid=HID,
    gblk=GBLK,
    nbuf=NBUF,
    msact=MSACT,
    prelu_mode=PRELU_MODE,
):
    npc = n_nodes // ncores
    ngroups = len(sched)
    segw = 8 * kmax
    nch = in_ch // P  # 2 channel halves

    nc = bacc.Bacc(
        "TRN2", target_bir_lowering=False, debug=False,
        num_swdge_queues=4, dynamic_dma_scratch_size=32768,
    )
    half = n_nodes // 2
    x_ds = [
        nc.dram_tensor(f"x{h}", [half, in_ch], BF16, kind="ExternalInput")
        for h in range(2)
    ]
    si_d = nc.dram_tensor(
        "idx16", [P, 2 * ngroups * segw], mybir.dt.int16, kind="ExternalInput"
    )
    ms_d = nc.dram_tensor("msel", [P, npairs * P], BF16, kind="ExternalInput")
    xs_d = nc.dram_tensor("xsall", [P, BPC * in_ch], BF16, kind="ExternalInput")
    wt_ds = [
        nc.dram_tensor(f"wt{h}", [P, hid], BF16, kind="ExternalInput")
        for h in range(nch)
    ]
    bs_d = nc.dram_tensor("bias", [1, hid], F32R, kind="ExternalInput")
    sdg_d = nc.dram_tensor("sdg", [1, BPC * P], F32R, kind="ExternalInput")
    idr_d = nc.dram_tensor("idr", [P, P], BF16, kind="ExternalInput")
    ct_d = nc.dram_tensor(
        "cnts", [1, 2 * ngroups], mybir.dt.int32, kind="ExternalInput"
    )
    dv_d = nc.dram_tensor("dinvc", [P, BPC], F32, kind="ExternalInput")
    adv_d = nc.dram_tensor("adinvc", [P, BPC], F32, kind="ExternalInput")
    out_d = nc.dram_tensor("out", [npc, hid], F32, kind="ExternalOutput")

    with tile.TileContext(nc) as tc, ExitStack() as ctx:
        const = ctx.enter_context(tc.tile_pool(name="const", bufs=1))
        gxp = ctx.enter_context(tc.tile_pool(name="gx", bufs=nbuf))
        mselp = ctx.enter_context(tc.tile_pool(name="msel", bufs=5))
        psA = ctx.enter_context(tc.tile_pool(name="psA", bufs=2, space="PSUM"))
        hps = ctx.enter_context(tc.tile_pool(name="hps", bufs=3, space="PSUM"))
        aS = ctx.enter_context(tc.tile_pool(name="aS", bufs=4))
        outp = ctx.enter_context(tc.tile_pool(name="outp", bufs=3))

        si_t = const.tile([P, 2 * ngroups * segw], mybir.dt.int16)
        nc.sync.dma_start(out=si_t[:], in_=si_d.ap())
        xs_t = const.tile([P, BPC * in_ch], BF16)
        nc.sync.dma_start(out=xs_t[:], in_=xs_d.ap())
        wt_t = []
        for h in range(nch):
            w = const.tile([P, hid], BF16, name=f"wt_t{h}")
            nc.sync.dma_start(out=w[:], in_=wt_ds[h].ap())
            wt_t.append(w)
        bs_t = const.tile([1, hid], F32R)
        nc.sync.dma_start(out=bs_t[:], in_=bs_d.ap())
        sdg_t = const.tile([1, BPC * P], F32R)
        nc.sync.dma_start(out=sdg_t[:], in_=sdg_d.ap())
        idr_t = const.tile([P, P], BF16)
        nc.sync.dma_start(out=idr_t[:], in_=idr_d.ap())
        ct_t = const.tile([1, 2 * ngroups], mybir.dt.int32)
        nc.sync.dma_start(out=ct_t[:], in_=ct_d.ap())
        greg = nc.alloc_register(mybir.EngineType.Pool, "gcnt")
        dv_t = const.tile([P, BPC], F32)
        nc.sync.dma_start(out=dv_t[:], in_=dv_d.ap())
        adv_t = const.tile([P, BPC], F32)
        nc.sync.dma_start(out=adv_t[:], in_=adv_d.ap())

        gather_qn = 0
        for g in range(ngroups):
            sg = sched[g]
            blocks = list(range(g * gblk, min((g + 1) * gblk, BPC)))
            nbl = len(blocks)
            # gathers (lo/hi)
            gxs = []
            for h in range(2):
                nidx = sg["nidx"][h]
                gx = gxp.tile(
                    [P, kmax * in_ch], BF16, tag=f"gx{h}", name=f"gx{h}_{g}"
                )
                kk = nidx // P
                soff = (g * 2 + h) * segw
                nc.gpsimd.reg_load(greg, ct_t[0:1, g * 2 + h : g * 2 + h + 1])
                nc.gpsimd.dma_gather(
                    gx[:, : kk * in_ch].rearrange("p (k d) -> p k d", d=in_ch),
                    x_ds[h].ap(),
                    si_t[:, soff : soff + 8 * kk],
                    nidx,
                    greg,
                    in_ch,
                    queue_num=gather_qn % 4,
                    single_packet=False,
                )
                gather_qn += 1
                gxs.append(gx)
            # AT accumulators per (local block, ch half)
            AT = {}
            started = {}
            for bl in range(nbl):
                t = psA.tile([P, nch * P], F32, tag=f"at{bl}", name=f"at{bl}_{g}")
                for hh in range(nch):
                    AT[(bl, hh)] = t[:, hh * P : (hh + 1) * P]
                started[bl] = False
            # msel tiles for this group arrive via one HWDGE DMA
            pc0, pc1 = sg["pc0"], sg["pc1"]
            npg = pc1 - pc0
            ms_t = mselp.tile(
                [P, npg * P], BF16, tag="msg", name=f"msg_{g}"
            )
            nc.sync.dma_start(
                out=ms_t[:], in_=ms_d.ap()[:, pc0 * P : pc1 * P]
            )
            # chunk matmuls
            for h in range(2):
                for (j, bl, col) in sg["mm"][h]:
                    ci = col - pc0
                    gx = gxs[h]
                    for hh in range(nch):
                        nc.tensor.matmul(
                            AT[(bl, hh)],
                            lhsT=gx[:, j * in_ch + hh * P : j * in_ch + (hh + 1) * P],
                            rhs=ms_t[:, ci * P : (ci + 1) * P],
                            start=not started[bl],
                            stop=False,
                        )
                        started[bl] = True
            # per block: self-loop (stop), AT->SBUF, weight mm, PReLU, store
            for bl, b in enumerate(blocks):
                ns = min(P, npc - b * P)
                ats = []
                for hh in range(nch):
                    nc.tensor.matmul(
                        AT[(bl, hh)],
                        lhsT=xs_t[:, b * in_ch + hh * P : b * in_ch + (hh + 1) * P],
                        rhs=idr_t[:],
                        start=not started[bl],
                        stop=(hh == nch - 1),
                    )
                    started[bl] = True
                for hh in range(nch):
                    a = aS.tile([P, P], BF16, tag=f"ats{hh}", name=f"ats{hh}_{b}")
                    nc.scalar.copy(a[:], AT[(bl, hh)])
                    ats.append(a)
                Hp = hps.tile([P, hid], F32, tag="hp", name=f"hp_{b}")
                for hh in range(nch):
                    nc.tensor.matmul(
                        Hp[:ns],
                        lhsT=ats[hh][:, :ns],
                        rhs=wt_t[hh][:],
                        start=(hh == 0),
                        stop=False,
                    )
                nc.tensor.matmul(
                    Hp[:ns],
                    lhsT=sdg_t[:, b * P : b * P + ns],
                    rhs=bs_t[:],
                    start=False,
                    stop=True,
                )
                os_ = outp.tile([P, hid], F32, tag="os", name=f"os_{b}")
                if prelu_mode == "act":
                    # PReLU(dinv*H) in one ACT op (exact for any alpha)
                    nc.scalar.activation(
                        out=os_[:ns],
                        in_=Hp[:ns],
                        func=mybir.ActivationFunctionType.Prelu,
                        scale=dv_t[:ns, b : b + 1],
                        alpha=float(alpha),
                    )
                else:
                    # fallback: max(dinv*H, alpha*dinv*H) (0<=alpha<=1)
                    t1 = outp.tile([P, hid], F32, tag="t1", name=f"t1_{b}")
                    t2 = outp.tile([P, hid], F32, tag="t2", name=f"t2_{b}")
                    nc.scalar.activation(
                        out=t1[:ns],
                        in_=Hp[:ns],
                        func=mybir.ActivationFunctionType.Copy,
                        scale=dv_t[:ns, b : b + 1],
                    )
                    nc.scalar.activation(
                        out=t2[:ns],
                        in_=Hp[:ns],
                        func=mybir.ActivationFunctionType.Copy,
                        scale=adv_t[:ns, b : b + 1],
                    )
                    nc.vector.tensor_tensor(
                        out=os_[:ns], in0=t1[:ns], in1=t2[:ns],
                        op=mybir.AluOpType.max,
                    )
                row0 = b * P
                nc.sync.dma_start(
                    out=out_d.ap()[row0 : row0 + ns, :], in_=os_[:ns, :]
                )
    nc.compile()
    return nc


def _make_in_maps(x, weight, bias, idx16, msel, cnts, dinv, alpha, ncores=NCORES):
    x = np.asarray(x, dtype=np.float32)
    w = np.asarray(weight, dtype=np.float32)
    n = x.shape[0]
    half = n // 2
    in_ch = x.shape[1]
    hid = w.shape[0]
    npc = n // ncores
    bf = mybir.dt.np(BF16)

    xp = x * dinv[:, None]  # prescaled by dinv[src]
    xlo = np.ascontiguousarray(xp[:half].astype(bf))
    xhi = np.ascontiguousarray(xp[half:].astype(bf))
    wts = {
        f"wt{h}": np.ascontiguousarray(w[:, h * P : (h + 1) * P].T.astype(bf))
        for h in range(in_ch // P)
    }
    bias_row = np.asarray(bias, dtype=np.float32).reshape(1, hid)
    sdeg = (1.0 / dinv).astype(np.float32)  # sqrt(deg)

    in_maps = []
    for k in range(ncores):
        sl = slice(k * npc, (k + 1) * npc)
        # xsall[d, b*256 + c] = (dinv*x)[k*npc + b*128 + d, c]
        xs = np.zeros((BPC * P, in_ch), np.float32)
        xs[:npc] = xp[sl]
        xsall = np.ascontiguousarray(
            xs.reshape(BPC, P, in_ch).transpose(1, 0, 2).reshape(P, BPC * in_ch)
        ).astype(bf)
        dv = np.zeros((BPC * P,), np.float32)
        dv[:npc] = dinv[sl]
        dvc = np.ascontiguousarray(dv.reshape(BPC, P).T)
        sdg = np.zeros((1, BPC * P), np.float32)
        sdg[0, :npc] = sdeg[sl]
        m = {
            "x0": xlo,
            "x1": xhi,
            "idx16": np.ascontiguousarray(idx16[k]),
            "cnts": np.ascontiguousarray(cnts[k : k + 1]),
            "msel": np.ascontiguousarray(msel[k].astype(bf)),
            "xsall": xsall,
            "bias": bias_row,
            "sdg": sdg,
            "idr": np.eye(P, dtype=bf),
            "dinvc": dvc,
            "adinvc": np.ascontiguousarray(dvc * float(alpha)),
        }
        m.update(wts)
        in_maps.append(m)
    return in_maps


# Results of the last kernel() call, for the test harness.
LAST_RESULTS = None


def kernel(x, edge_index, weight, bias, prelu_a):
    global LAST_RESULTS
    trace = os.environ.get("GCN_TRACE", "0") == "1"

    sched, kmax, npairs, idx16, msel, cnts, dinv = _preprocess(edge_index)
    alpha = float(np.asarray(prelu_a).ravel()[0])
    nc = _build_program(sched, kmax, npairs, alpha)
    in_maps = _make_in_maps(x, weight, bias, idx16, msel, cnts, dinv, alpha)

    res = bass_utils.run_bass_kernel_spmd(
        nc, in_maps, core_ids=list(range(NCORES)), trace=trace
    )
    LAST_RESULTS = res
    out = np.concatenate([res.results[k]["out"] for k in range(NCORES)], axis=0)
    return out.astype(np.float32)


# revision 15
# speedup vs baseline: 1.8536x; 1.4647x over previous
"""GCN encoder (GCNConv + PReLU) as a Bass/Tile kernel on 8 Trainium2 NeuronCores.

Math (matches PyG GCNConv with self-loops + symmetric norm, then PReLU):
    deg[i]  = in-degree of i over dst (+1 self loop)
    dinv    = 1/sqrt(deg)
    agg[d]  = sum_{e:(s->d)} dinv[s]*dinv[d] * x[s] + dinv[d]^2 * x[d]
    out     = PReLU(agg @ W.T + bias)

Distribution: dst-node sharding, core k owns nodes [k*6250, (k+1)*6250).

Key structure (v2):
  - x is host-prescaled by dinv[src] and stored bf16 in two 25000-row halves
    (int16 gather indices). Per dst-group (GBLK blocks of 128 dst), edges are
    packed contiguously (block-major) and fetched with dma_gather; trailing
    index slots are -1 so the Q7 descriptor generator trims them (desc-gen
    cost == real edge count, no cross-core padding cost).
  - Msel[e, d] = (iota == dstl[e]) is a pure 0/1 selection built with a single
    DVE is_equal op (bf16), optionally some on ACT via relu(1 - |iota-dstl|).
  - The scatter-add accumulates the TRANSPOSED aggregate directly:
        AT_h[c, d] += gx[e, c]^T @ Msel[e, d]   (PE, bf16, PSUM f32)
    so no PE transpose pass is needed before the weight matmul.
  - Self-loop: AT_h += xsall[d, c]^T @ I via one identity matmul per half
    (xsall = dinv * x own rows, dense HWDGE load).
  - H[d, :] = sum_h AT_h^T @ W_h + sqrtdeg[d] * bias  (PSUM accumulation;
    the sqrtdeg row makes the bias exact after the final dinv[d] scaling).
  - out = Prelu(H * dinv[d]) in ONE scalar-engine activation (parametric_relu
    is resident in every activation table; alpha is the PReLU scalar).

Env knobs: GCN_GBLK (2), GCN_NBUF (8), GCN_MSACT (0..8: of every 8 msel
builds, this many go to ACT), GCN_PRELU=act|max.
"""

import os
import numpy as np

import concourse.bass as bass
import concourse.tile as tile
from concourse import bacc, mybir, bass_utils
from contextlib import ExitStack

# Problem shape (fixed by the harness contract).
N_NODES = 50000
N_EDGES = 400000
IN_CH = 256
HID = 512
NCORES = 8
NPC = N_NODES // NCORES  # dst nodes owned per core
P = 128
BPC = (NPC + P - 1) // P  # dst blocks per core (49)

F32 = mybir.dt.float32
F32R = mybir.dt.float32r
BF16 = mybir.dt.bfloat16

GBLK = int(os.environ.get("GCN_GBLK", "2"))
FULLGEN = os.environ.get("GCN_FULLGEN", "0") == "1"
NBUF = int(os.environ.get("GCN_NBUF", "8"))
MSACT = int(os.environ.get("GCN_MSACT", "0"))
PRELU_MODE = os.environ.get("GCN_PRELU", "act")
MS_DT = os.environ.get("GCN_MS_DT", "fp8")  # fp8 | bf16
OUT_DT = os.environ.get("GCN_OUT_DT", "bf16")  # bf16 | f32


def _preprocess(edge_index, n_nodes=N_NODES, ncores=NCORES, gblk=GBLK, nbuf=NBUF):
    """Group non-self edges by (core, dst-group, src-half), pack block-major.

    Returns (sched, kmax, npairs, idx16, dstl, cnts, dinv):
      sched: per group g: dict(kk=[Klo,Khi] static chunk counts,
             nidx=[...] static gather num_idxs,
             mm={h: [(j, local_b, paircol), ...]} static matmul schedule)
      idx16: [ncores, 128, 2*ngroups*8*kmax] int16 gather indices
      dstl:  [ncores, 128, npairs] f32; dst-local in block or -1
      dinv:  [n_nodes] f32
    """
    npc = n_nodes // ncores
    half = n_nodes // 2
    ngroups = (BPC + gblk - 1) // gblk
    src = np.asarray(edge_index[0]).astype(np.int64).ravel()
    dst = np.asarray(edge_index[1]).astype(np.int64).ravel()
    deg = np.bincount(dst, minlength=n_nodes).astype(np.float32) + 1.0
    dinv = (1.0 / np.sqrt(deg)).astype(np.float32)

    core = dst // npc
    dloc = dst - core * npc
    blk = dloc // P
    grp = blk // gblk
    hi = (src >= half).astype(np.int64)

    # sort edges by (core, grp, hi, blk, src)
    key = (((core * ngroups + grp) * 2 + hi) * BPC + blk) * (half + 1) + (
        src - hi * half
    )
    order = np.argsort(key, kind="stable")
    src_s, core_s = src[order], core[order]
    grp_s, hi_s, blk_s = grp[order], hi[order], blk[order]
    dll_s = (dloc[order] - blk_s * P).astype(np.int64)  # dst local in block

    # counts per (core, grp, hi) and per (core, grp, hi, blk)
    cgh = (core_s * ngroups + grp_s) * 2 + hi_s
    n_cgh = ncores * ngroups * 2
    cnt_cgh = np.bincount(cgh, minlength=n_cgh).reshape(ncores, ngroups, 2)
    cghb = cgh * BPC + blk_s
    cnt_cghb = np.bincount(cghb, minlength=n_cgh * BPC).reshape(
        ncores, ngroups, 2, BPC
    )

    kgh = -(-cnt_cgh.max(axis=0) // P)  # [ngroups, 2] static chunk counts
    kgh = np.maximum(kgh, 1)
    kmax = int(kgh.max())

    # rank of each edge within its (core, grp, hi) segment
    seg_start = np.zeros(n_cgh + 1, np.int64)
    seg_start[1:] = np.cumsum(cnt_cgh.ravel())
    rank = np.arange(len(order)) - seg_start[cgh]

    # static matmul schedule: union over cores of blocks present in chunk j
    # block b of (g,h) occupies ranks [bs, bs+cnt) -> chunks bs//P .. (bs+cnt-1)//P
    bstart = np.cumsum(cnt_cghb, axis=3) - cnt_cghb  # [nc, ng, 2, BPC]
    sched = []
    npairs = 0
    paircol = {}
    for g in range(ngroups):
        blocks = list(range(g * gblk, min((g + 1) * gblk, BPC)))
        mm = {0: [], 1: []}
        for h in range(2):
            k = int(kgh[g, h])
            for j in range(k):
                for b in blocks:
                    bl = b - g * gblk
                    lo = bstart[:, g, h, b]
                    cnt = cnt_cghb[:, g, h, b]
                    # does block b intersect chunk j on any core?
                    inter = np.any(
                        (cnt > 0) & (lo < (j + 1) * P) & (lo + cnt > j * P)
                    )
                    if inter:
                        paircol[(g, h, j, b)] = npairs
                        mm[h].append((j, bl, npairs))
                        npairs += 1
        nidx = [
            (kmax if g < nbuf else int(kgh[g, h])) * P for h in range(2)
        ]
        pc1 = npairs
        sched.append(
            {"kk": [int(kgh[g, 0]), int(kgh[g, 1])], "nidx": nidx, "mm": mm,
             "pc0": pc1 - len(mm[0]) - len(mm[1]), "pc1": pc1}
        )

    # per-core data arrays
    segw = 8 * kmax  # idx16 columns per (g,h) segment
    fill = 0 if FULLGEN else -1
    idx16 = np.full((ncores, 16, 2 * ngroups * segw), fill, np.int16)
    # first-rotation groups: pad with 0 (valid) so gx pool buffers are
    # fully written once before trimmed gathers leave stale tails
    for g in range(min(nbuf, ngroups)):
        for h in range(2):
            s = (g * 2 + h) * segw
            idx16[:, :, s : s + segw] = 0
    seg = (grp_s * 2 + hi_s) * segw
    col = seg + (rank // 16)
    row = rank % 16
    idx16[core_s, row, col] = (src_s - hi_s * half).astype(np.int16)

    # msel one-hot tiles: edge at rank r -> chunk j=r//P, slot p=r%P;
    # pair (g,h,j,blk) -> column pc*P + dst_local
    j_s = rank // P
    p_s = rank % P
    pc = np.array(
        [paircol[(g, h, j, b)] for g, h, j, b in zip(grp_s, hi_s, j_s, blk_s)],
        dtype=np.int64,
    )
    msel = np.zeros((ncores, P, npairs * P), np.float32)
    msel[core_s, p_s, pc * P + dll_s] = 1.0

    idx16 = np.tile(idx16, (1, 8, 1))  # replicate to 128 partitions

    # runtime gather counts per (core, g, h): real edges (trailing -1 slots
    # are trimmed by the Q7 desc generator); first-rotation groups gather
    # their full static extent (padded with index 0)
    cnts = np.empty((ncores, 2 * ngroups), np.int32)
    for g in range(ngroups):
        for h in range(2):
            if g < nbuf:
                cnts[:, g * 2 + h] = kmax * P
            elif FULLGEN:
                cnts[:, g * 2 + h] = int(
                    np.ceil(cnt_cgh[:, g, h].max() / P)
                ) * P
            else:
                cnts[:, g * 2 + h] = cnt_cgh[:, g, h]
    return sched, kmax, npairs, idx16, msel, cnts, dinv


def _build_program(
    sched,
    kmax,
    npairs,
    alpha,
    n_nodes=N_NODES,
    ncores=NCORES,
    in_ch=IN_CH,
    hid=HID,
    gblk=GBLK,
    nbuf=NBUF,
    msact=MSACT,
    prelu_mode=PRELU_MODE,
):
    npc = n_nodes // ncores
    ngroups = len(sched)
    segw = 8 * kmax
    nch = in_ch // P  # 2 channel halves

    nc = bacc.Bacc(
        "TRN2", target_bir_lowering=False, debug=False,
        num_swdge_queues=4, dynamic_dma_scratch_size=32768,
    )
    half = n_nodes // 2
    x_ds = [
        nc.dram_tensor(f"x{h}", [half, in_ch], BF16, kind="ExternalInput")
        for h in range(2)
    ]
    si_d = nc.dram_tensor(
        "idx16", [P, 2 * ngroups * segw], mybir.dt.int16, kind="ExternalInput"
    )
    ms_dt = mybir.dt.float8e4 if MS_DT == "fp8" else BF16
    out_dt = BF16 if OUT_DT == "bf16" else F32
    ms_d = nc.dram_tensor("msel", [P, npairs * P], ms_dt, kind="ExternalInput")
    xs_d = nc.dram_tensor("xsall", [P, BPC * in_ch], BF16, kind="ExternalInput")
    wt_ds = [
        nc.dram_tensor(f"wt{h}", [P, hid], BF16, kind="ExternalInput")
        for h in range(nch)
    ]
    bs_d = nc.dram_tensor("bias", [1, hid], F32R, kind="ExternalInput")
    sdg_d = nc.dram_tensor("sdg", [1, BPC * P], F32R, kind="ExternalInput")
    idr_d = nc.dram_tensor("idr", [P, P], BF16, kind="ExternalInput")
    ct_d = nc.dram_tensor(
        "cnts", [1, 2 * ngroups], mybir.dt.int32, kind="ExternalInput"
    )
    dv_d = nc.dram_tensor("dinvc", [P, BPC], F32, kind="ExternalInput")
    adv_d = nc.dram_tensor("adinvc", [P, BPC], F32, kind="ExternalInput")
    out_d = nc.dram_tensor("out", [npc, hid], out_dt, kind="ExternalOutput")

    with tile.TileContext(nc) as tc, ExitStack() as ctx:
        const = ctx.enter_context(tc.tile_pool(name="const", bufs=1))
        gxp = ctx.enter_context(tc.tile_pool(name="gx", bufs=nbuf))
        mselp = ctx.enter_context(tc.tile_pool(name="msel", bufs=5))
        psA = ctx.enter_context(tc.tile_pool(name="psA", bufs=2, space="PSUM"))
        hps = ctx.enter_context(tc.tile_pool(name="hps", bufs=3, space="PSUM"))
        aS = ctx.enter_context(tc.tile_pool(name="aS", bufs=4))
        outp = ctx.enter_context(tc.tile_pool(name="outp", bufs=3))

        si_t = const.tile([P, 2 * ngroups * segw], mybir.dt.int16)
        nc.sync.dma_start(out=si_t[:], in_=si_d.ap())
        xs_t = const.tile([P, BPC * in_ch], BF16)
        nc.sync.dma_start(out=xs_t[:], in_=xs_d.ap())
        wt_t = []
        for h in range(nch):
            w = const.tile([P, hid], BF16, name=f"wt_t{h}")
            nc.sync.dma_start(out=w[:], in_=wt_ds[h].ap())
            wt_t.append(w)
        bs_t = const.tile([1, hid], F32R)
        nc.sync.dma_start(out=bs_t[:], in_=bs_d.ap())
        sdg_t = const.tile([1, BPC * P], F32R)
        nc.sync.dma_start(out=sdg_t[:], in_=sdg_d.ap())
        idr_t = const.tile([P, P], BF16)
        nc.sync.dma_start(out=idr_t[:], in_=idr_d.ap())
        ct_t = const.tile([1, 2 * ngroups], mybir.dt.int32)
        nc.sync.dma_start(out=ct_t[:], in_=ct_d.ap())
        greg = nc.alloc_register(mybir.EngineType.Pool, "gcnt")
        dv_t = const.tile([P, BPC], F32)
        nc.sync.dma_start(out=dv_t[:], in_=dv_d.ap())
        adv_t = const.tile([P, BPC], F32)
        nc.sync.dma_start(out=adv_t[:], in_=adv_d.ap())

        gather_qn = 0
        for g in range(ngroups):
            sg = sched[g]
            blocks = list(range(g * gblk, min((g + 1) * gblk, BPC)))
            nbl = len(blocks)
            # gathers (lo/hi)
            gxs = []
            for h in range(2):
                nidx = sg["nidx"][h]
                gx = gxp.tile(
                    [P, kmax * in_ch], BF16, tag=f"gx{h}", name=f"gx{h}_{g}"
                )
                kk = nidx // P
                soff = (g * 2 + h) * segw
                nc.gpsimd.reg_load(greg, ct_t[0:1, g * 2 + h : g * 2 + h + 1])
                nc.gpsimd.dma_gather(
                    gx[:, : kk * in_ch].rearrange("p (k d) -> p k d", d=in_ch),
                    x_ds[h].ap(),
                    si_t[:, soff : soff + 8 * kk],
                    nidx,
                    greg,
                    in_ch,
                    queue_num=gather_qn % 4,
                    single_packet=False,
                )
                gather_qn += 1
                gxs.append(gx)
            # AT accumulators per (local block, ch half)
            AT = {}
            started = {}
            for bl in range(nbl):
                t = psA.tile([P, nch * P], F32, tag=f"at{bl}", name=f"at{bl}_{g}")
                for hh in range(nch):
                    AT[(bl, hh)] = t[:, hh * P : (hh + 1) * P]
                started[bl] = False
            # msel tiles for this group arrive via one HWDGE DMA
            pc0, pc1 = sg["pc0"], sg["pc1"]
            npg = pc1 - pc0
            ms_t = mselp.tile(
                [P, npg * P], ms_dt, tag="msg", name=f"msg_{g}"
            )
            nc.sync.dma_start(
                out=ms_t[:], in_=ms_d.ap()[:, pc0 * P : pc1 * P]
            )
            # chunk matmuls
            for h in range(2):
                for (j, bl, col) in sg["mm"][h]:
                    ci = col - pc0
                    gx = gxs[h]
                    for hh in range(nch):
                        nc.tensor.matmul(
                            AT[(bl, hh)],
                            lhsT=gx[:, j * in_ch + hh * P : j * in_ch + (hh + 1) * P],
                            rhs=ms_t[:, ci * P : (ci + 1) * P],
                            start=not started[bl],
                            stop=False,
                        )
                        started[bl] = True
            # per block: self-loop (stop), AT->SBUF, weight mm, PReLU, store
            for bl, b in enumerate(blocks):
                ns = min(P, npc - b * P)
                ats = []
                for hh in range(nch):
                    nc.tensor.matmul(
                        AT[(bl, hh)],
                        lhsT=xs_t[:, b * in_ch + hh * P : b * in_ch + (hh + 1) * P],
                        rhs=idr_t[:],
                        start=not started[bl],
                        stop=(hh == nch - 1),
                    )
                    started[bl] = True
                for hh in range(nch):
                    a = aS.tile([P, P], BF16, tag=f"ats{hh}", name=f"ats{hh}_{b}")
                    nc.scalar.copy(a[:], AT[(bl, hh)])
                    ats.append(a)
                Hp = hps.tile([P, hid], F32, tag="hp", name=f"hp_{b}")
                for hh in range(nch):
                    nc.tensor.matmul(
                        Hp[:ns],
                        lhsT=ats[hh][:, :ns],
                        rhs=wt_t[hh][:],
                        start=(hh == 0),
                        stop=False,
                    )
                nc.tensor.matmul(
                    Hp[:ns],
                    lhsT=sdg_t[:, b * P : b * P + ns],
                    rhs=bs_t[:],
                    start=False,
                    stop=True,
                )
                os_ = outp.tile([P, hid], out_dt, tag="os", name=f"os_{b}")
                if prelu_mode == "act":
                    # PReLU(dinv*H) in one ACT op (exact for any alpha)
                    nc.scalar.activation(
                        out=os_[:ns],
                        in_=Hp[:ns],
                        func=mybir.ActivationFunctionType.Prelu,
                        scale=dv_t[:ns, b : b + 1],
                        alpha=float(alpha),
                    )
                else:
                    # fallback: max(dinv*H, alpha*dinv*H) (0<=alpha<=1)
                    t1 = outp.tile([P, hid], F32, tag="t1", name=f"t1_{b}")
                    t2 = outp.tile([P, hid], F32, tag="t2", name=f"t2_{b}")

                    nc.scalar.activation(
                        out=t1[:ns],
                        in_=Hp[:ns],
                        func=mybir.ActivationFunctionType.Copy,
                        scale=dv_t[:ns, b : b + 1],
                    )
                    nc.scalar.activation(
                        out=t2[:ns],
                        in_=Hp[:ns],
                        func=mybir.ActivationFunctionType.Copy,
                        scale=adv_t[:ns, b : b + 1],
                    )
                    nc.vector.tensor_tensor(
                        out=os_[:ns], in0=t1[:ns], in1=t2[:ns],
                        op=mybir.AluOpType.max,
                    )
                row0 = b * P
                nc.sync.dma_start(
                    out=out_d.ap()[row0 : row0 + ns, :], in_=os_[:ns, :]
                )
    nc.compile()
    return nc


def _make_in_maps(x, weight, bias, idx16, msel, cnts, dinv, alpha, ncores=NCORES):
    x = np.asarray(x, dtype=np.float32)
    w = np.asarray(weight, dtype=np.float32)
    n = x.shape[0]
    half = n // 2
    in_ch = x.shape[1]
    hid = w.shape[0]
    npc = n // ncores
    bf = mybir.dt.np(BF16)

    xp = x * dinv[:, None]  # prescaled by dinv[src]
    xlo = np.ascontiguousarray(xp[:half].astype(bf))
    xhi = np.ascontiguousarray(xp[half:].astype(bf))
    wts = {
        f"wt{h}": np.ascontiguousarray(w[:, h * P : (h + 1) * P].T.astype(bf))
        for h in range(in_ch // P)
    }
    bias_row = np.asarray(bias, dtype=np.float32).reshape(1, hid)
    sdeg = (1.0 / dinv).astype(np.float32)  # sqrt(deg)

    in_maps = []
    for k in range(ncores):
        sl = slice(k * npc, (k + 1) * npc)
        # xsall[d, b*256 + c] = (dinv*x)[k*npc + b*128 + d, c]
        xs = np.zeros((BPC * P, in_ch), np.float32)
        xs[:npc] = xp[sl]
        xsall = np.ascontiguousarray(
            xs.reshape(BPC, P, in_ch).transpose(1, 0, 2).reshape(P, BPC * in_ch)
        ).astype(bf)
        dv = np.zeros((BPC * P,), np.float32)
        dv[:npc] = dinv[sl]
        dvc = np.ascontiguousarray(dv.reshape(BPC, P).T)
        sdg = np.zeros((1, BPC * P), np.float32)
        sdg[0, :npc] = sdeg[sl]
        m = {
            "x0": xlo,
            "x1": xhi,
            "idx16": np.ascontiguousarray(idx16[k]),
            "cnts": np.ascontiguousarray(cnts[k : k + 1]),
            "msel": np.ascontiguousarray(
                msel[k].astype(mybir.dt.np(mybir.dt.float8e4) if MS_DT == "fp8" else bf)
            ),
            "xsall": xsall,
            "bias": bias_row,
            "sdg": sdg,
            "idr": np.eye(P, dtype=bf),
            "dinvc": dvc,
            "adinvc": np.ascontiguousarray(dvc * float(alpha)),
        }
        m.update(wts)
        in_maps.append(m)
    return in_maps


# Results of the last kernel() call, for the test harness.
LAST_RESULTS = None


def kernel(x, edge_index, weight, bias, prelu_a):
    global LAST_RESULTS
    trace = os.environ.get("GCN_TRACE", "0") == "1"

    sched, kmax, npairs, idx16, msel, cnts, dinv = _preprocess(edge_index)
    alpha = float(np.asarray(prelu_a).ravel()[0])
    nc = _build_program(sched, kmax, npairs, alpha)
    in_maps = _make_in_maps(x, weight, bias, idx16, msel, cnts, dinv, alpha)

    res = bass_utils.run_bass_kernel_spmd(
        nc, in_maps, core_ids=list(range(NCORES)), trace=trace
    )
    LAST_RESULTS = res
    out = np.concatenate(
        [np.asarray(res.results[k]["out"], dtype=np.float32) for k in range(NCORES)],
        axis=0,
    )
    return out
